# revision 20
# baseline (speedup 1.0000x reference)
"""Trainium2 Bass kernel for a 2-layer GCN + 2-layer MLP (gnn_message_passing).

Model (see reference):
    h1 = relu(GCNConv(x;  W1, b1))       # symmetric-normalized, self-loops
    h2 = relu(GCNConv(h1; W2, b2))
    h3 = relu(h2 @ Wl1 + bl1)
    y  = h3 @ Wl2 + bl2                  # [N, 1]

Distribution: nodes (and the edges whose *destination* they are) are
partitioned across 8 NeuronCores.  Each core computes the scaled feature
table T = (dinv * h) @ W for its node shard; shards are AllGathered (the
"halo exchange"), and each core aggregates messages for its own
destination nodes with SWDGE dma_gather (64-byte rows from a
256B-strided table) plus one-hot matmul scatter-add in PSUM.

SWDGE descriptor generation is the bottleneck: ~2.3 ns/descriptor
effective across the 4 queues (one Q7 cpu pair each).  The design
minimizes descriptor count (MD=112 dst-blocks cut slot padding to ~10%)
and keeps the generators busy continuously (a 32 KiB descriptor
carveout fits two full per-chunk batches per ring, so chunk g+1's
generation overlaps chunk g's DMA drain).

Per chunk (448 dsts), everything else hides under the ~70us of
descriptor generation: one-hot tiles are built just-in-time per source
range on the DVE, the scatter matmuls run on the otherwise-idle PE, and
the next layer's table projection (or the final MLP) is folded into the
chunk loop so no full-width intermediate lives in SBUF.

All symmetric-normalization (dinv) factors are folded into the host-side
x pre-scale and the per-chunk fix stage; matmul outputs are produced in
transposed [H, dst] orientation so the fix is a single full-rate DVE
multiply plus a Scalar-engine ReLU+bias, with no on-chip transposes.
"""

import math
import sys

import numpy as np

sys.path.insert(0, "/opt/trn_rl_repo")

import concourse.bass as bass
import concourse.mybir as mybir
import concourse.ap_utils as ap_utils
import concourse.tile as tile
from concourse import bacc
from concourse._compat import exact_div
from concourse.bass_utils import run_bass_kernel_spmd


def dma_gather_raw(eng, out_ap, in_ap, idxs_ap, num_idxs, num_idxs_reg,
                   elem_size, elem_step, single_packet=False, queue_num=0):
    """gathered = in[idxs, :elem_size]; rows strided elem_step elements.

    Clone of BassGpSimd.dma_gather's HBM path minus the
    `elem_size_bytes % 256 == 0` restriction (the Q7 ucode only requires the
    row STRIDE to be a 256-byte multiple; payload bytes are free)."""
    assert idxs_ap.dtype == mybir.dt.int16
    assert in_ap.dtype == out_ap.dtype
    dt_size = mybir.dt.size(in_ap.dtype)
    assert ap_utils.ap_is_contiguous(out_ap.ap[1:])
    assert ap_utils.ap_is_contiguous(idxs_ap.ap[1:])
    assert in_ap.ap[-1][1] == out_ap.ap[-1][1] == elem_size
    assert out_ap.ap[0][1] * out_ap.ap[1][1] == ((num_idxs + 127) // 128) * 128
    assert in_ap.ap[0][0] == elem_step
    stride_bytes_256 = exact_div(elem_step * dt_size, 256)
    assert stride_bytes_256 < 256

    _in_ap = eng.lower_ap_dma(in_ap, for_custom_bir_dma=True)
    _idxs_ap = eng.lower_ap(idxs_ap)
    _out_ap = eng.lower_ap(out_ap)
    return eng.add_instruction(
        mybir.InstDMAGatherAnt(
            name=eng.bass.get_next_instruction_name(),
            ins=[*_in_ap, _idxs_ap,
                 eng.lower_val_access(eng.to_reg(num_idxs_reg))],
            outs=[_out_ap],
            transpose=False,
            num_idxs=num_idxs,
            elem_size=elem_size,
            stride_bytes_256=stride_bytes_256,
            gen_mode=0,
            single_packet=single_packet,
            queue_num=queue_num,
            sbuf_tokens_per_rank=0,
            sbuf_free_dim_per_rank=0,
            sbuf_free_dim_pad_per_rank=0,
            sbuf_byte_offset=0,
        )
    )

FP16 = mybir.dt.float16
FP32 = mybir.dt.float32
INT16 = mybir.dt.int16
Alu = mybir.AluOpType
Act = mybir.ActivationFunctionType

N_CORES = 8
MD = 112           # dst-block size (free-dim block of the one-hot matmul)
N_RANGE = 4        # src-id ranges (int16 gather indices: table <= 32768 rows)
CB = 4             # dst-blocks per gather chunk
ROWW = 128         # table row stride in fp16 elements (256 B, SWDGE req)
HID = 32


class Cfg:
    def __init__(self, n_nodes, in_ch, hid, tbc):
        self.in_ch = in_ch
        self.hid = hid
        self.n_cores = N_CORES
        npc = math.ceil(n_nodes / N_CORES)
        lcm = math.lcm(128, MD * CB)
        npc = ((npc + lcm - 1) // lcm) * lcm
        self.npc = npc
        self.npad = npc * N_CORES
        assert self.npad % N_RANGE == 0
        self.rng_sz = self.npad // N_RANGE
        assert self.rng_sz <= 32768
        self.nb = npc // MD
        self.ng = self.nb // CB
        self.ntile128 = npc // 128
        # tbc [nb, N_RANGE]: 128-slot tiles per (block, range) bucket,
        # shared by all cores (max).  CSR offsets over the slot grid:
        self.tbc = tbc
        # per (g, r): columns and chunk-local range offsets
        self.cgr = np.zeros((self.ng, N_RANGE), np.int64)
        for g in range(self.ng):
            for r in range(N_RANGE):
                self.cgr[g, r] = tbc[g * CB:(g + 1) * CB, r].sum()
        self.gc = self.cgr.sum(1)                    # columns per chunk
        self.gcol0 = np.concatenate([[0], np.cumsum(self.gc)])
        self.ntt = int(self.gcol0[-1])               # total grid columns
        self.gcmax = int(self.gc.max())
        self.cgrmax = int(self.cgr.max())
        # per-chunk idx-slab width (columns of int16): one band per queue
        self.wg = (self.cgr * 8).max(1)
        self.wg0 = np.concatenate([[0], np.cumsum(self.wg)])
        self.wtot = int(self.wg0[-1])


def host_prep(x, edge_index, W1, b1, W2, b2, Wl1, bl1, Wl2, bl2):
    n = x.shape[0]
    src = np.asarray(edge_index[0], dtype=np.int64)
    dst = np.asarray(edge_index[1], dtype=np.int64)
    in_ch = x.shape[1]
    hid = W1.shape[1]

    lcm = math.lcm(128, MD * CB)
    npc = ((math.ceil(n / N_CORES) + lcm - 1) // lcm) * lcm
    npad = npc * N_CORES
    rng_sz = npad // N_RANGE
    nb = npc // MD

    # reference degree: in-degree (by dst) + 1 for the self loop
    deg = (np.bincount(dst, minlength=npad) + 1.0).astype(np.float32)
    dinv = (1.0 / np.sqrt(deg)).astype(np.float32)

    # self-loops are NOT gathered: dinv_d^2 * (hW)[d] is folded into the
    # fix stage via the transposed table (t1T/t2T)
    src_a, dst_a = src, dst
    owner = dst_a // npc

    # Table row layout after the two half-AllGathers (each contiguous):
    # [all cores' first half-shards; all cores' second half-shards].
    hc = npc // 2
    s_own = src_a // npc
    s_loc = src_a % npc
    s_hi = s_loc >= hc
    s_row = s_own * hc + (s_loc - s_hi * hc) + s_hi * (npad // 2)

    # per (core, block, range) edge lists
    blk = (dst_a - owner * npc) // MD
    rng = src_a % N_RANGE
    key = ((owner * nb) + blk) * N_RANGE + rng
    order = np.argsort(key, kind="stable")
    src_a, dst_a, key = src_a[order], dst_a[order], key[order]
    s_row = s_row[order]
    cnts = np.bincount(key, minlength=N_CORES * nb * N_RANGE).reshape(
        N_CORES, nb, N_RANGE)
    tbc = np.maximum(((cnts + 127) // 128).max(0), 1)   # [nb, N_RANGE]

    cfg = Cfg(n, in_ch, hid, tbc)

    # chunk-local CSR column offsets: per (g, r, bl)
    colof = np.zeros((cfg.ng, N_RANGE, CB), np.int64)
    for g in range(cfg.ng):
        base = 0
        for r in range(N_RANGE):
            colof[g, r] = base + np.concatenate(
                [[0], np.cumsum(tbc[g * CB:(g + 1) * CB, r])])[:-1]
            base += cfg.cgr[g, r]

    ofs = np.concatenate([[0], np.cumsum(cnts.reshape(-1))])
    gidx_all, dstloc_all = [], []
    for c in range(N_CORES):
        gsl = np.zeros((128, cfg.ntt), dtype=np.int16)
        dloc = np.full((128, cfg.ntt), 10000.0, dtype=np.float16)
        for b in range(nb):
            g, bl = b // CB, b % CB
            for r in range(N_RANGE):
                k = (c * nb + b) * N_RANGE + r
                s, e = ofs[k], ofs[k + 1]
                cnt = e - s
                if cnt == 0:
                    continue
                col0 = cfg.gcol0[g] + colof[g, r, bl]
                sl = np.arange(cnt)
                p_i, t_i = sl % 128, sl // 128
                gsl[p_i, col0 + t_i] = (s_row[s:e] // N_RANGE).astype(np.int16)
                dloc[p_i, col0 + t_i] = (dst_a[s:e] - c * npc - b * MD
                                         ).astype(np.float16)
        # per-chunk idx slab [128, wg[g]]: queue q's stream at partitions
        # [32q, 32q+32) (two 16-partition-wrapped replicas); q = (g + r) % 4
        idxw = np.zeros((128, cfg.wtot), dtype=np.int16)
        for g in range(cfg.ng):
            w0 = cfg.wg0[g]
            cbase = cfg.gcol0[g]
            for r in range(N_RANGE):
                cgr = cfg.cgr[g, r]
                flat = gsl[:, cbase:cbase + cgr].T.reshape(-1)  # (col, p)
                w = flat.reshape(-1, 16).T                      # [16, cgr*8]
                q = (g + r) % 4
                idxw[32 * q:32 * q + 16, w0:w0 + cgr * 8] = w
                idxw[32 * q + 16:32 * q + 32, w0:w0 + cgr * 8] = w
                cbase += cgr
        gidx_all.append(idxw)
        dstloc_all.append(dloc)

    # host-applied dinv[src] pre-scale of x; per-dst dinv table for the fix
    xf = np.zeros((npad, in_ch), dtype=np.float32)
    xf[:n] = np.asarray(x, dtype=np.float32)
    xf *= dinv[:, None]
    dinvex_all, xT_all = [], []
    for c in range(N_CORES):
        d = dinv[c * npc:(c + 1) * npc].astype(np.float16)
        dinvex_all.append(np.ascontiguousarray(np.tile(d[None, :], (hid, 1))))
        xT_all.append(np.ascontiguousarray(
            xf[c * npc:(c + 1) * npc].T).astype(np.float16))

    # one-hot compare pattern, (column, dst) order: iota[p, t*MD+d] = d
    iota = np.zeros((128, cfg.cgrmax * MD), dtype=np.float16)
    for dd in range(MD):
        iota[:, dd::MD] = float(dd)

    consts = {
        "W1": np.asarray(W1, np.float16),
        "W2": np.asarray(W2, np.float16),
        "Wl1": np.asarray(Wl1, np.float16),
        "Wl2": np.asarray(Wl2, np.float16),
        "b1": np.asarray(b1, np.float32).reshape(hid, 1),
        "b2": np.asarray(b2, np.float32).reshape(hid, 1),
        "bl1": np.asarray(bl1, np.float32).reshape(hid, 1),
        "bl2": np.asarray(bl2, np.float32).reshape(1, 1),
        "iotaM": iota,
    }
    in_maps = []
    for c in range(N_CORES):
        m = dict(consts)
        m["xT"] = xT_all[c]
        m["gidx"] = gidx_all[c]
        m["dstloc"] = dstloc_all[c]
        m["dinvex"] = dinvex_all[c]
        in_maps.append(m)
    return cfg, in_maps


def build_program(cfg: Cfg):
    # 32 KiB/partition descriptor carveout = 2048 descs per (engine, dir)
    # ring: two full per-chunk gather batches fit, so chunk g+1's descriptor
    # generation overlaps chunk g's DMA drain instead of stalling on
    # await_space (the 16 KiB default fits only ~1.7 batches).
    nc = bacc.Bacc("TRN2", target_bir_lowering=False, num_swdge_queues=4,
                   dynamic_dma_scratch_size=32768)
    H, NPC, NG = cfg.hid, cfg.npc, cfg.ng
    CW = CB * MD                       # chunk width in dsts (448)
    tbc = cfg.tbc

    xT_d = nc.dram_tensor("xT", [cfg.in_ch, NPC], FP16, kind="ExternalInput")
    W1_d = nc.dram_tensor("W1", [cfg.in_ch, H], FP16, kind="ExternalInput")
    W2_d = nc.dram_tensor("W2", [H, H], FP16, kind="ExternalInput")
    Wl1_d = nc.dram_tensor("Wl1", [H, H], FP16, kind="ExternalInput")
    Wl2_d = nc.dram_tensor("Wl2", [H, 1], FP16, kind="ExternalInput")
    b1_d = nc.dram_tensor("b1", [H, 1], FP32, kind="ExternalInput")
    b2_d = nc.dram_tensor("b2", [H, 1], FP32, kind="ExternalInput")
    bl1_d = nc.dram_tensor("bl1", [H, 1], FP32, kind="ExternalInput")
    bl2_d = nc.dram_tensor("bl2", [1, 1], FP32, kind="ExternalInput")
    gidx_d = nc.dram_tensor("gidx", [128, cfg.wtot], INT16,
                            kind="ExternalInput")
    dstloc_d = nc.dram_tensor("dstloc", [128, cfg.ntt], FP16,
                              kind="ExternalInput")
    dinvex_d = nc.dram_tensor("dinvex", [H, NPC], FP16, kind="ExternalInput")
    iota_d = nc.dram_tensor("iotaM", [128, cfg.cgrmax * MD], FP16,
                            kind="ExternalInput")
    y_d = nc.dram_tensor("y", [NPC], FP32, kind="ExternalOutput")

    t1s_d = nc.dram_tensor("t1s", [NPC, H], FP16)
    t2s_d = nc.dram_tensor("t2s", [NPC, H], FP16)
    t1T_d = nc.dram_tensor("t1T", [H, NPC], FP16)   # own-shard table, transposed
    t2T_d = nc.dram_tensor("t2T", [H, NPC], FP16)
    t1c_d = nc.dram_tensor("t1c", [cfg.npad, H], FP16, addr_space="Shared")
    t2c_d = nc.dram_tensor("t2c", [cfg.npad, H], FP16, addr_space="Shared")

    iota_s = nc.alloc_sbuf_tensor("iota_s", [128, cfg.cgrmax * MD], FP16).ap()
    dinvex_s = nc.alloc_sbuf_tensor("dinvex_s", [H, NPC], FP16).ap()
    W2_s = nc.alloc_sbuf_tensor("W2_s", [H, H], FP16).ap()
    Wl1_s = nc.alloc_sbuf_tensor("Wl1_s", [H, H], FP16).ap()
    Wl2_s = nc.alloc_sbuf_tensor("Wl2_s", [H, 1], FP16).ap()
    b1_s = nc.alloc_sbuf_tensor("b1_s", [H, 1], FP32).ap()
    b2_s = nc.alloc_sbuf_tensor("b2_s", [H, 1], FP32).ap()
    bl1_s = nc.alloc_sbuf_tensor("bl1_s", [H, 1], FP32).ap()
    bl2_s = nc.alloc_sbuf_tensor("bl2_s", [1, 1], FP32).ap()

    cc_sem = nc.alloc_semaphore("cc_sem")

    def agg_chunk(g, bigpool, pool, apsum, tc_d, tT_d, b_s, last):
        """One chunk of GCN aggregation; returns the fixed [H, CW] tile."""
        cbase = int(cfg.gcol0[g])
        gc = int(cfg.gc[g])
        idxb = bigpool.tile([128, int(cfg.wg.max())], INT16, tag="idx")
        nc.sync.dma_start(
            idxb[:, :int(cfg.wg[g])],
            gidx_d[:, int(cfg.wg0[g]):int(cfg.wg0[g] + cfg.wg[g])])
        dlb = bigpool.tile([128, cfg.gcmax], FP16, tag="dloc")
        nc.sync.dma_start(dlb[:, :gc], dstloc_d[:, cbase:cbase + gc])
        sfT = bigpool.tile([H, CW], FP16, tag="selfT")
        nc.sync.dma_start(sfT[:], tT_d[:, g * CW:(g + 1) * CW])
        mts = []
        for r in range(N_RANGE):
            cgr = int(cfg.cgr[g, r])
            mt = bigpool.tile([128, cfg.cgrmax, H], FP16, tag=f"msg{r}")
            tq = tc_d[:].rearrange("(q f) h -> q (f h)", f=N_RANGE)
            dma_gather_raw(
                nc.gpsimd, mt[:, :cgr, :],
                tq[0:cfg.npad // N_RANGE, r * H:(r + 1) * H],
                idxb[:, :cgr * 8],
                cgr * 128, cgr * 128, H, ROWW,
                queue_num=(g + r) % 4)
            mts.append(mt)
        # Each range accumulates into its own single-bank PSUM tile so every
        # (range, block) accumulation group is a consecutive matmul run
        # (interleaving start/stop groups within a region corrupts PSUM);
        # the fix stage sums the four partials on the DVE.
        psr = [apsum.tile([H, CW], FP32, tag=f"agg{r}", name=f"psr{r}")
               for r in range(N_RANGE)]
        co = 0
        for r in range(N_RANGE):
            cgr = int(cfg.cgr[g, r])
            oh = bigpool.tile([128, cfg.cgrmax, MD], FP16, tag="oh")
            dl3 = dlb[:, co:co + cgr].rearrange(
                "p (t a) -> p t a", a=1).to_broadcast([128, cgr, MD])
            nc.vector.tensor_tensor(
                oh[:, :cgr, :], dl3,
                iota_s[:, :cgr * MD].rearrange("p (t d) -> p t d", d=MD),
                Alu.is_equal)
            for bl in range(CB):
                b = g * CB + bl
                # range-local column of (bl, t=0):
                crl = int(tbc[g * CB:g * CB + bl, r].sum())
                ntr = int(tbc[b, r])
                for t in range(ntr):
                    nc.tensor.matmul(psr[r][:, bl * MD:(bl + 1) * MD],
                                     mts[r][:, crl + t, :],
                                     oh[:, crl + t, :],
                                     start=(t == 0), stop=(t == ntr - 1))
            co += cgr
        tmp = pool.tile([H, CW], FP32, tag="fix")
        dv = dinvex_s[:, g * CW:(g + 1) * CW]
        # self-loop message is the table row itself (carries dinv_d already)
        nc.scalar.activation(tmp[:], psr[0][:], Act.Copy)
        for r in range(1, N_RANGE):
            nc.vector.tensor_tensor(tmp[:], tmp[:], psr[r][:], Alu.add)
        nc.vector.tensor_tensor(tmp[:], tmp[:], sfT[:], Alu.add)
        nc.vector.tensor_tensor(tmp[:], tmp[:], dv, Alu.mult)
        ht = pool.tile([H, CW], FP16, tag="ht")
        nc.scalar.activation(ht[:], tmp[:], Act.Relu, bias=b_s[:, 0:1])
        if not last:
            nc.vector.tensor_tensor(ht[:], ht[:], dv, Alu.mult)
        return ht


    # ---------------- Phase 1: constants, T1 ----------------
    with tile.TileContext(nc) as tc:
        with tc.tile_pool(name="p1", bufs=2) as pool, \
             tc.tile_pool(name="p1ps", bufs=2, space="PSUM") as psum:
            nc.sync.dma_start(iota_s[:], iota_d[:])
            nc.sync.dma_start(dinvex_s[:], dinvex_d[:])
            nc.sync.dma_start(W2_s[:], W2_d[:])
            nc.sync.dma_start(Wl1_s[:], Wl1_d[:])
            nc.sync.dma_start(Wl2_s[:], Wl2_d[:])
            nc.sync.dma_start(b1_s[:], b1_d[:])
            nc.sync.dma_start(b2_s[:], b2_d[:])
            nc.sync.dma_start(bl1_s[:], bl1_d[:])
            nc.sync.dma_start(bl2_s[:], bl2_d[:])

            xT = pool.tile([cfg.in_ch, NPC], FP16)
            nc.sync.dma_start(xT[:], xT_d[:])
            W1 = pool.tile([cfg.in_ch, H], FP16)
            nc.sync.dma_start(W1[:], W1_d[:])
            for j0 in range(0, cfg.ntile128, 4):
                jn = min(4, cfg.ntile128 - j0)
                ps = psum.tile([128, 4, H], FP32, tag="tbl_ps")
                for jj in range(jn):
                    j = j0 + jj
                    nc.tensor.matmul(ps[:, jj, :],
                                     xT[0:cfg.in_ch, j * 128:(j + 1) * 128],
                                     W1[:], start=True, stop=True)
                ts = pool.tile([128, 4, H], FP16, tag="tbl_sb")
                nc.scalar.activation(ts[:, :jn, :], ps[:, :jn, :], Act.Copy)
                nc.sync.dma_start(
                    t1s_d[j0 * 128:(j0 + jn) * 128, :].rearrange(
                        "(j p) h -> p j h", p=128),
                    ts[:, :jn, :])
            for s0 in range(0, NPC, 512):
                sl = min(512, NPC - s0)
                psK = psum.tile([H, 512], FP32, tag="t1t_ps")
                nc.tensor.matmul(psK[:, :sl], W1[:], xT[:, s0:s0 + sl],
                                 start=True, stop=True)
                tk = pool.tile([H, 512], FP16, tag="t1t_sb")
                nc.scalar.activation(tk[:, :sl], psK[:, :sl], Act.Copy)
                nc.sync.dma_start(t1T_d[:, s0:s0 + sl], tk[:, :sl])

    HC = NPC // 2
    nc.gpsimd.collective_compute(
        "AllGather", Alu.bypass, replica_groups=[list(range(N_CORES))],
        ins=[t1s_d[0:HC, :]],
        outs=[t1c_d[0:cfg.npad // 2, :]]).then_inc(cc_sem, 1)
    nc.gpsimd.collective_compute(
        "AllGather", Alu.bypass, replica_groups=[list(range(N_CORES))],
        ins=[t1s_d[HC:NPC, :]],
        outs=[t1c_d[cfg.npad // 2:cfg.npad, :]]).then_inc(cc_sem, 1)
    nc.gpsimd.wait_ge(cc_sem, 2)

    # ---------------- Phase 2: aggregate L1, T2 (interleaved) ----------------
    # split into halves: the first half-table's AllGather overlaps the
    # second half's chunk loop
    def phase2_half(h0, h1):
        with tile.TileContext(nc) as tc:
            with tc.tile_pool(name=f"p2b{h0}", bufs=3) as bigpool, \
                 tc.tile_pool(name=f"p2{h0}", bufs=3) as pool, \
                 tc.tile_pool(name=f"p2a{h0}", bufs=1, space="PSUM") as apsum, \
                 tc.tile_pool(name=f"p2ps{h0}", bufs=2, space="PSUM") as psum:
                for g in range(h0, h1):
                    ht = agg_chunk(g, bigpool, pool, apsum, t1c_d, t1T_d,
                                   b1_s, False)
                    ps = psum.tile([MD, CB, H], FP32, tag="tbl_ps")
                    for jj in range(CB):
                        nc.tensor.matmul(ps[:, jj, :],
                                         ht[:, jj * MD:(jj + 1) * MD],
                                         W2_s[:], start=True, stop=True)
                    ts = pool.tile([MD, CB, H], FP16, tag="tbl_sb")
                    nc.scalar.activation(ts[:], ps[:], Act.Copy)
                    nc.sync.dma_start(
                        t2s_d[g * CW:(g + 1) * CW, :].rearrange(
                            "(j p) h -> p j h", p=MD),
                        ts[:])
                    psX = psum.tile([H, CW], FP32, tag="t2t_ps")
                    nc.tensor.matmul(psX[:], W2_s[:], ht[:],
                                     start=True, stop=True)
                    tx = pool.tile([H, CW], FP16, tag="t2t_sb")
                    nc.scalar.activation(tx[:], psX[:], Act.Copy)
                    nc.sync.dma_start(t2T_d[:, g * CW:(g + 1) * CW], tx[:])

    NH = NG // 2
    HROWS = NH * CW
    assert HROWS == NPC // 2
    phase2_half(0, NH)
    nc.gpsimd.collective_compute(
        "AllGather", Alu.bypass, replica_groups=[list(range(N_CORES))],
        ins=[t2s_d[0:HROWS, :]],
        outs=[t2c_d[0:cfg.npad // 2, :]]).then_inc(cc_sem, 1)
    phase2_half(NH, NG)
    nc.gpsimd.collective_compute(
        "AllGather", Alu.bypass, replica_groups=[list(range(N_CORES))],
        ins=[t2s_d[HROWS:NPC, :]],
        outs=[t2c_d[cfg.npad // 2:cfg.npad, :]]).then_inc(cc_sem, 1)
    nc.gpsimd.wait_ge(cc_sem, 4)

    # ---------------- Phase 3: aggregate L2, MLP (interleaved) ----------------
    with tile.TileContext(nc) as tc:
        with tc.tile_pool(name="p3b", bufs=3) as bigpool, \
             tc.tile_pool(name="p3", bufs=3) as pool, \
             tc.tile_pool(name="p3a", bufs=1, space="PSUM") as apsum, \
             tc.tile_pool(name="p3ps", bufs=2, space="PSUM") as psum:
            y2 = y_d[:].rearrange("(a n) -> a n", a=1)
            for g in range(NG):
                ht = agg_chunk(g, bigpool, pool, apsum, t2c_d, t2T_d, b2_s, True)
                zp = psum.tile([H, CW], FP32, tag="zps")
                nc.tensor.matmul(zp[:], Wl1_s[:], ht[:],
                                 start=True, stop=True)
                zt = pool.tile([H, CW], FP16, tag="zt")
                nc.scalar.activation(zt[:], zp[:], Act.Relu,
                                     bias=bl1_s[:, 0:1])
                yp = psum.tile([1, CW], FP32, tag="yps")
                nc.tensor.matmul(yp[:], Wl2_s[:], zt[:],
                                 start=True, stop=True)
                ys = pool.tile([1, CW], FP32, tag="ysl")
                nc.scalar.activation(ys[:], yp[:],
                                     Act.Identity, bias=bl2_s[:, 0:1])
                nc.sync.dma_start(y2[:, g * CW:(g + 1) * CW], ys[:])

    nc.compile()
    return nc


_CACHE = {}


def _get_program(key, cfg):
    if key not in _CACHE:
        _CACHE[key] = build_program(cfg)
    return _CACHE[key]


def kernel(x, edge_index, W1, b1, W2, b2, Wl1, bl1, Wl2, bl2):
    x = np.asarray(x)
    n = x.shape[0]
    cfg, in_maps = host_prep(x, edge_index, W1, b1, W2, b2, Wl1, bl1, Wl2, bl2)
    key = (n, cfg.in_ch, cfg.hid, cfg.tbc.tobytes())
    nc = _get_program(key, cfg)
    res = run_bass_kernel_spmd(nc, in_maps, list(range(N_CORES)))
    ys = [res.results[c]["y"].reshape(-1) for c in range(N_CORES)]
    y = np.concatenate(ys)[:n].astype(np.float32)
    return y.reshape(n, 1)
